# revision 1
# baseline (speedup 1.0000x reference)
"""GCN message-passing (gather + segment-sum) on 8 TRN2 NeuronCores.

out[v] = sum over edges (u -> v) of features[u]

Strategy (dst-sharded, self-contained per core — no collectives):
  - 8 cores each own a 12544-node dst range (8 x 12544 = 100352 >= 100000).
  - Features live in DRAM as a padded table of 256-byte rows ([*, 64] f32,
    payload in [:, :32]) split into 4 chunks of 25088 rows + one zero row
    each, so each chunk is addressable by int16 dma_gather indices.
  - Per (core, section=src-chunk): edges are scheduled by destination;
    dst nodes are ranked by in-degree (descending).  Rank r maps to
    accumulator slot (partition r%128, group r//128); each group of 128
    ranks shares a run length R_g (cross-core max => one static NEFF).
    A node's message slots are consecutive columns of its partition.
  - dma_gather (GPSIMD SWDGE, 4 queues round-robin, 1024-idx batches)
    fills staging tiles [128, cols, 64]; padding slots gather a zero row.
  - DVE tensor_reduce sums each run level (strided X-reduce) into acc
    tiles [128, 98, 64] (payload [:, :, :32]).
  - dma_scatter_add (batched like the gathers) adds acc rows into
    out[node_id]: the scatter applies the rank->node permutation AND
    merges the 4 sections via the DMA CCE.  Indices are unique within a
    section; sections are serialized against each other.
  - Host concatenates the 8 core outputs and trims to 100000 rows.
"""

import numpy as np

import concourse.bass as bass
import concourse.mybir as mybir
from concourse import bacc
from concourse.bass_utils import run_bass_kernel_spmd

# problem constants (hardcoded per harness contract)
N_NODES = 100000
N_EDGES = 1600000
D = 32

P = 128
N_CORES = 8
NODES_PER_CORE = 12544           # 98 * 128
N_GROUPS = NODES_PER_CORE // P   # 98
N_SEC = 4
CHUNK = 25088                    # nodes per src chunk
TROW = CHUNK + 1                 # +1 zero row per chunk
ZROW = CHUNK                     # local index of the zero row
ELEM = 64                        # table row: 64 f32 = 256 B
BATCH = 1024                     # idxs per SWDGE prep (ring cap ~1024-1536)
BCOLS = BATCH // P               # 8 columns per gather batch
NQ = 4                           # SWDGE queues
BLK_TARGET = 96                  # target columns per staging block
SC_PER_SEC = (NODES_PER_CORE + BATCH - 1) // BATCH  # 13 scatter batches/section
IDXW = NODES_PER_CORE // 16      # 784 wrapped scatter-idx columns per section


def _wrap_idx(stream):
    """[n] int stream -> [128, n//16] int16, replicated across the 8 Q7 cores."""
    n = len(stream)
    w = np.asarray(stream, np.int16).reshape(n // 16, 16).T  # pos i -> (i%16, i//16)
    return np.tile(w, (8, 1))


def _build_schedule(src32, dst32):
    core = dst32 // NODES_PER_CORE
    ldst = dst32 - core * NODES_PER_CORE
    sec = src32 // CHUNK
    lsrc = src32 - sec * CHUNK

    flat = (core * N_SEC + sec) * NODES_PER_CORE + ldst
    cnt = np.bincount(flat, minlength=N_CORES * N_SEC * NODES_PER_CORE)
    cnt = cnt.reshape(N_CORES, N_SEC, NODES_PER_CORE).astype(np.int32)

    order = np.argsort(-cnt, axis=2, kind="stable")       # rank -> node
    scnt = -np.sort(-cnt, axis=2)                         # degree at rank (desc)

    # shared per-section group run length: max over cores at each group head
    R_all = scnt[:, :, 0::P].max(axis=0)                  # [N_SEC, 98]

    # rank of each node per (core, sec)
    rank = np.empty_like(order)
    ar = np.arange(NODES_PER_CORE)
    for c in range(N_CORES):
        for s in range(N_SEC):
            rank[c, s, order[c, s]] = ar

    blocks = []          # [s] -> list of (col0, ncols, levels)
    cols = []            # [s] -> padded column count
    colmap_all = []      # [s][g] -> first column of group g
    for s in range(N_SEC):
        R = R_all[s]
        lv = []
        g = 0
        while g < N_GROUPS and R[g] > 0:
            g1 = g
            while g1 + 1 < N_GROUPS and R[g1 + 1] == R[g]:
                g1 += 1
            lv.append((g, g1 + 1, int(R[g])))
            g = g1 + 1

        blks = []
        colmap = np.zeros(N_GROUPS, np.int64)
        state = {"col": 0, "levels": [], "col0": 0, "cols": 0}

        def close_block():
            if not state["levels"]:
                return
            pad = (-state["cols"]) % BCOLS
            state["cols"] += pad
            blks.append((state["col0"], state["cols"], state["levels"]))
            state["col"] = state["col0"] + state["cols"]
            state["col0"] = state["col"]
            state["cols"] = 0
            state["levels"] = []

        for (g0, g1, R_lv) in lv:
            g = g0
            while g < g1:
                room = BLK_TARGET - state["cols"]
                if R_lv > room and state["cols"] > 0:
                    close_block()
                    continue
                take = min(max(1, room // R_lv), g1 - g)
                lcol = state["cols"]
                state["levels"].append((g, g + take, R_lv, lcol))
                for gg in range(g, g + take):
                    colmap[gg] = state["col0"] + lcol + (gg - g) * R_lv
                state["cols"] += take * R_lv
                g += take
                if state["cols"] >= BLK_TARGET:
                    close_block()
        close_block()
        blocks.append(blks)
        cols.append(state["col"])
        colmap_all.append(colmap)

    total_cols = int(sum(cols))
    sec_colbase = np.cumsum([0] + cols)[:-1].astype(np.int64)

    gidx = []
    sidx = []
    for c in range(N_CORES):
        stream = np.full(P * total_cols, ZROW, np.int64)
        for s in range(N_SEC):
            m = (core == c) & (sec == s)
            r = rank[c, s][ldst[m]]
            v = lsrc[m]
            o = np.argsort(r, kind="stable")
            r = r[o]
            v = v[o]
            starts = np.searchsorted(r, ar)
            k = np.arange(len(r)) - starts[r]
            g = r // P
            p = r % P
            j = colmap_all[s][g] + k                 # column within section
            pos = P * (sec_colbase[s] + j) + p
            stream[pos] = v
        gidx.append(_wrap_idx(stream))
        sid = np.concatenate([order[c, s] for s in range(N_SEC)])
        sidx.append(_wrap_idx(sid))

    return {
        "blocks": blocks,
        "cols": cols,
        "sec_colbase": sec_colbase,
        "total_cols": total_cols,
        "gidx": gidx,
        "sidx": sidx,
    }


def _build_nc(sched, reps=1, skip_reduce=False, skip_scatter=False, dbg_fix=()):
    """reps>1 repeats the whole pipeline (for timing; output is then wrong)."""
    blocks = sched["blocks"]
    sec_colbase = sched["sec_colbase"]
    total_cols = sched["total_cols"]

    blkmax = max(ncols for s in range(N_SEC) for (_, ncols, _) in blocks[s])
    nb_per_rep = sum(len(blocks[s]) for s in range(N_SEC))

    nc = bacc.Bacc("TRN2", target_bir_lowering=False, debug=False,
                   num_devices=N_CORES, num_swdge_queues=NQ)

    feat = nc.dram_tensor("feat", [N_SEC * TROW, ELEM], mybir.dt.float32, kind="ExternalInput")
    gidx = nc.dram_tensor("gidx", [P, 8 * total_cols], mybir.dt.int16, kind="ExternalInput")
    sidx = nc.dram_tensor("sidx", [P, N_SEC * IDXW], mybir.dt.int16, kind="ExternalInput")
    out = nc.dram_tensor("out", [NODES_PER_CORE, ELEM], mybir.dt.float32, kind="ExternalOutput")

    gidx_t = nc.alloc_sbuf_tensor("gidx_t", [P, 8 * total_cols], mybir.dt.int16)
    sidx_t = nc.alloc_sbuf_tensor("sidx_t", [P, N_SEC * IDXW], mybir.dt.int16)
    stage = [nc.alloc_sbuf_tensor(f"stage{i}", [P, blkmax * ELEM], mybir.dt.float32) for i in range(2)]
    acc = [nc.alloc_sbuf_tensor(f"acc{i}", [P, N_GROUPS * ELEM], mybir.dt.float32) for i in range(2)]

    # ---- flat block list over reps: (gs, bi, s, col0, ncols, levels) ----
    blist = []
    for rep in range(reps):
        for s in range(N_SEC):
            for (col0, ncols, levels) in blocks[s]:
                blist.append((rep * N_SEC + s, len(blist), s, col0, ncols, levels))
    # last block index (global) of each global section
    last_bi_of_gs = {}
    first_bi_of_gs = {}
    for (gs, bi, s, col0, ncols, levels) in blist:
        last_bi_of_gs[gs] = bi
        first_bi_of_gs.setdefault(gs, bi)

    # ---- SWDGE entry plan (issue order) ----
    entries = []
    for (gs, bi, s, col0, ncols, levels) in blist:
        for k in range(ncols // BCOLS):
            entries.append(("g", gs, bi, s, int(sec_colbase[s] + col0 + k * BCOLS), k * BCOLS))
        if bi == last_bi_of_gs[gs] and not skip_scatter:
            left = NODES_PER_CORE
            kk = 0
            while left > 0:
                n = min(BATCH, left)
                entries.append(("s", gs, s, kk, n))
                left -= n
                kk += 1

    qnext = [None] * NQ
    gq_cnt = [0] * NQ
    sc_idx = 0
    plan = []
    gcum_of_block = {}
    run = [0] * NQ
    qi = 0
    for e in entries:
        q = qi % NQ
        qi += 1
        plan.append((e, q, qnext[q]))
        if e[0] == "g":
            gq_cnt[q] += 1
            qnext[q] = ("g", q, gq_cnt[q])
            run[q] += 1
            gcum_of_block[e[2]] = tuple(run)
        else:
            sc_idx += 1
            qnext[q] = ("s", sc_idx)
    n_scatters = sc_idx
    qcum = []
    lastc = (0,) * NQ
    for bi in range(len(blist)):
        lastc = gcum_of_block.get(bi, lastc)
        qcum.append(lastc)

    with (
        nc.Block() as block,
        nc.semaphore("ld") as ld,
        nc.semaphore("q0") as q0s,
        nc.semaphore("q1") as q1s,
        nc.semaphore("q2") as q2s,
        nc.semaphore("q3") as q3s,
        nc.semaphore("qp0") as qp0,
        nc.semaphore("qp1") as qp1,
        nc.semaphore("qp2") as qp2,
        nc.semaphore("qp3") as qp3,
        nc.semaphore("red") as red,
        nc.semaphore("sd") as sd,
    ):
        qdma = [q0s, q1s, q2s, q3s]
        qprep = [qp0, qp1, qp2, qp3]

        def emit_wait(g, tok):
            if tok is None:
                return
            if tok[0] == "g":
                g.wait_ge(qdma[tok[1]], 16 * tok[2])
            else:
                g.wait_ge(sd, 16 * tok[1])

        @block.gpsimd
        def _(g: bass.BassGpSimd):
            g.dma_start(out=gidx_t[:], in_=gidx[:]).then_inc(ld, 16)
            g.dma_start(out=sidx_t[:], in_=sidx[:]).then_inc(ld, 16)
            g.wait_ge(ld, 32)
            qprep_cnt = [0] * NQ
            seen_blocks = set()
            for (e, q, wait_tok) in plan:
                if e[0] == "g":
                    (_, gs, bi, s, gcol, lc) = e
                    if bi not in seen_blocks:
                        seen_blocks.add(bi)
                        if bi >= 2 and not skip_reduce:
                            g.wait_ge(red, bi - 1)   # staging buf bi-2 reduced
                    emit_wait(g, wait_tok)
                    if "stage" in dbg_fix:
                        bi_, lc_ = 0, 0
                    else:
                        bi_, lc_ = bi, lc
                    s_ = 0 if "table" in dbg_fix else s
                    gcol_ = 0 if "idx" in dbg_fix else gcol
                    g.dma_gather(
                        out_ap=stage[bi_ % 2].ap().rearrange("p (c e) -> p c e", e=ELEM)[:, lc_:lc_ + BCOLS, :],
                        in_ap=feat[s_ * TROW:(s_ + 1) * TROW, :],
                        idxs_ap=gidx_t[:, 8 * gcol_:8 * (gcol_ + BCOLS)],
                        num_idxs=BATCH,
                        num_idxs_reg=BATCH,
                        elem_size=ELEM,
                        prepare_only=True,
                        sem=qdma[q],
                        queue_num=q,
                    ).then_inc(qprep[q], 1)
                else:
                    (_, gs, s, kk, n) = e
                    if kk == 0 and not skip_reduce:
                        g.wait_ge(red, last_bi_of_gs[gs] + 1)   # acc complete
                        if gs > 0:
                            g.wait_ge(sd, 16 * gs * SC_PER_SEC)  # RMW safety
                    emit_wait(g, wait_tok)
                    g.dma_scatter_add(
                        out_ap=out[:],
                        in_ap=acc[gs % 2].ap().rearrange("p (ge e) -> p ge e", e=ELEM)[:, kk * BCOLS:kk * BCOLS + (n + P - 1) // P, :],
                        idxs_ap=sidx_t[:, s * IDXW + kk * (BATCH // 16): s * IDXW + kk * (BATCH // 16) + (n // 16)],
                        num_idxs=n,
                        num_idxs_reg=n,
                        elem_size=ELEM,
                        prepare_only=True,
                        sem=sd,
                        queue_num=q,
                    ).then_inc(qprep[q], 1)
                qprep_cnt[q] += 1
                g.wait_ge(qprep[q], qprep_cnt[q])
                g.trigger_dma(count=1, queue_num=q)
            if n_scatters:
                g.wait_ge(sd, 16 * n_scatters)
            for q in range(NQ):
                if gq_cnt[q]:
                    g.wait_ge(qdma[q], 16 * gq_cnt[q])

        @block.vector
        def _(v: bass.BassEngine):
            if skip_reduce:
                return
            for (gs, bi, s, col0, ncols, levels) in blist:
                if bi == first_bi_of_gs[gs]:
                    if gs >= 2:
                        v.wait_ge(sd, 16 * (gs - 1) * SC_PER_SEC)
                    v.memset(acc[gs % 2].ap(), 0.0)
                for q in range(NQ):
                    if qcum[bi][q] > 0:
                        v.wait_ge(qdma[q], 16 * qcum[bi][q])
                stage_ap = stage[bi % 2].ap().rearrange("p (c e) -> p c e", e=ELEM)
                acc_ap = acc[gs % 2].ap().rearrange("p (ge e) -> p ge e", e=ELEM)
                last = None
                for (g0, g1, R, lcol) in levels:
                    src = stage_ap[:, lcol:lcol + (g1 - g0) * R, 0:D] \
                        .rearrange("p (gr r) d -> p gr d r", r=R)
                    last = v.tensor_reduce(
                        out=acc_ap[:, g0:g1, 0:D],
                        in_=src,
                        axis=mybir.AxisListType.X,
                        op=mybir.AluOpType.add,
                    )
                last.then_inc(red, 1)

    nc.compile()
    return nc


def _run(nc, in_maps):
    try:
        return run_bass_kernel_spmd(nc, in_maps, list(range(N_CORES)))
    except Exception:
        return run_bass_kernel_spmd(nc, in_maps, list(range(N_CORES)))


def _prep_inputs(features, src, dst):
    features = np.asarray(features, np.float32)
    src32 = np.asarray(src).astype(np.int32)
    dst32 = np.asarray(dst).astype(np.int32)
    sched = _build_schedule(src32, dst32)
    fpad = np.zeros((N_CORES * NODES_PER_CORE, D), np.float32)
    fpad[:N_NODES] = features
    tab = np.zeros((N_SEC * TROW, ELEM), np.float32)
    for s in range(N_SEC):
        tab[s * TROW:s * TROW + CHUNK, :D] = fpad[s * CHUNK:(s + 1) * CHUNK]
    in_maps = [
        {"feat": tab, "gidx": sched["gidx"][c], "sidx": sched["sidx"][c]}
        for c in range(N_CORES)
    ]
    return sched, in_maps


def kernel(features, src, dst):
    sched, in_maps = _prep_inputs(features, src, dst)
    nc = _build_nc(sched)
    res = _run(nc, in_maps)
    out = np.concatenate([res.results[c]["out"][:, :D] for c in range(N_CORES)], axis=0)
    return np.ascontiguousarray(out[:N_NODES])


if __name__ == "__main__":
    rng = np.random.default_rng(0)
    feats = rng.standard_normal((N_NODES, D)).astype(np.float32)
    src = rng.integers(0, N_NODES, N_EDGES).astype(np.int64)
    dst = rng.integers(0, N_NODES, N_EDGES).astype(np.int64)
    got = kernel(feats, src, dst)
    exp = np.zeros((N_NODES, D), np.float32)
    np.add.at(exp, dst, feats[src])
    err = np.linalg.norm(got - exp) / np.linalg.norm(exp)
    print("rel err:", err)



# revision 2
# speedup vs baseline: 9.9416x; 9.9416x over previous
"""GCN message-passing (gather + segment-sum) on TRN2.

out[v] = sum over edges (u -> v) of features[u]
(features [100000, 32] f32, 1.6M edges)

Empirical facts for this environment (axon-tunneled TRN2):
  - DMA throughput collapses when all 8 cores issue heavy descriptor
    traffic concurrently; 1-4 cores run ~2 orders of magnitude faster
    per descriptor.  So we use 4 cores.
  - dma_gather handles 64-byte elements fine (bass's 256B-multiple
    assert is a front-end restriction; a relaxed clone of the call is
    inlined below).  Row pitch must still be a 256B multiple.
  - Large DVE ops are nearly free; small instructions cost ~1-3us, so
    everything is batched (4096-idx gathers, merged reduce levels).

Design:
  - dst-sharded across 4 cores; core c owns local dst range of 25088.
  - Features bf16, packed 4 nodes per 256B table row: node u lives at
    row u//4, col 32*(u%4); one zero row for padding.  The table stays
    in DRAM; gathers move only 64B per edge.
  - 4 "band" streams per core (band = src%4 = the col offset of the
    gather's in_ap).  Per (core, band), dst slots are ranked by edge
    count (desc); groups of 128 ranks share a run length R
    (cross-core max -> one static NEFF serves all cores).  Whole groups
    are packed into 32-col gather batches.
  - SWDGE gathers (4096 idxs/batch, 4 queues, staging ring) feed DVE
    tensor_reduce (strided X) which writes bf16 partial sums
    acc_b[128, G*32] (slot rank r -> partition r%128, group r//128).
  - One plain DMA per band writes acc_b out; the host applies the
    rank->node permutations and sums the 4 band partials in f32.
"""

import numpy as np
import ml_dtypes

import concourse.bass as bass
import concourse.mybir as mybir
from concourse import ap_utils
from concourse import bacc
from concourse.bass import MemorySpace, exact_div
from concourse.bass_utils import run_bass_kernel_spmd

N_NODES = 100000
N_EDGES = 1600000
D = 32
P = 128
NPAD = 100352          # 784 * 128
NBAND = 4
TROW = 25089           # 25088 packed rows + 1 zero row
ZROW = 25088

NCORE = 4
BATCH = 4096
BCOLS = BATCH // P     # 32 gather cols per batch
IDXC = BATCH // 16     # 256 wrapped idx cols per batch
DSS = 49152
NSLOT = 12             # staging ring slots (each BCOLS cols)
IDXSLOT = 8            # gidx ring slots


def _dma_gather_raw(g, out_ap, in_ap, idxs_ap, num_idxs, num_idxs_reg,
                    elem_size, elem_step, sem, queue_num):
    """bass.BassGpSimd.dma_gather (non-transpose, HBM source,
    prepare_only) with the elem_size_bytes % 256 assert relaxed to 64."""
    self = g
    self._assert_queue_num(queue_num)
    assert idxs_ap.dtype == mybir.dt.int16
    assert in_ap.dtype == out_ap.dtype
    elem_size_bytes = elem_size * mybir.dt.size(in_ap.dtype)
    assert elem_size_bytes > 0 and elem_size_bytes % 64 == 0
    assert in_ap.space == MemorySpace.DRAM
    assert idxs_ap.space == MemorySpace.SBUF
    assert out_ap.space == MemorySpace.SBUF
    assert ap_utils.ap_is_contiguous(out_ap.ap[1:])
    assert ap_utils.ap_is_contiguous(idxs_ap.ap[1:])
    assert in_ap.ap[-1][1] == out_ap.ap[-1][1] == elem_size
    assert out_ap.ap[0][1] * out_ap.ap[1][1] == bass.round_up_to_multiple(
        num_idxs, 128)
    assert in_ap.ap[0][0] == elem_step
    stride_bytes = elem_step * mybir.dt.size(in_ap.dtype)
    stride_bytes_256 = exact_div(stride_bytes, 256)
    assert stride_bytes_256 < 256

    _in_ap = self.lower_ap_dma(in_ap, for_custom_bir_dma=True)
    inst = self.add_instruction(
        mybir.InstDMAGatherAnt(
            name=self.bass.get_next_instruction_name(),
            ins=[
                *_in_ap,
                self.lower_ap(idxs_ap),
                self.lower_val_access(self.to_reg(num_idxs_reg)),
            ],
            outs=[self.lower_ap(out_ap)],
            transpose=False,
            num_idxs=num_idxs,
            elem_size=elem_size,
            stride_bytes_256=stride_bytes_256,
            gen_mode=1,
            single_packet=False,
            queue_num=queue_num,
            sbuf_tokens_per_rank=0,
            sbuf_free_dim_per_rank=0,
            sbuf_free_dim_pad_per_rank=0,
            sbuf_byte_offset=0,
        )
    )
    inst.then_inc(sem, 16)
    return self._track_prepare_only(inst, queue_num)


def _wrap_idx(stream):
    n = len(stream)
    w = np.asarray(stream, np.int16).reshape(n // 16, 16).T
    return np.tile(w, (8, 1))


def _build_schedule(src32, dst32, ncore=NCORE):
    dpc = NPAD // ncore
    G = dpc // P
    core = dst32 // dpc
    ldst = dst32 - core * dpc
    band = src32 % NBAND
    row = src32 // NBAND

    flat = (core * NBAND + band) * dpc + ldst
    cnt = np.bincount(flat, minlength=ncore * NBAND * dpc)
    cnt = cnt.reshape(ncore, NBAND, dpc).astype(np.int32)

    order = np.argsort(-cnt, axis=2, kind="stable")     # rank -> ldst
    scnt = -np.sort(-cnt, axis=2)

    # shared group run lengths: cross-core max at group heads
    R = scnt[:, :, 0::P].max(axis=0)                    # [NBAND, G]
    assert R.max() <= BCOLS, f"R.max()={R.max()} exceeds batch cols {BCOLS}"

    rank = np.empty_like(order)
    ar = np.arange(dpc)
    for c in range(ncore):
        for b in range(NBAND):
            rank[c, b, order[c, b]] = ar

    # pack rank-groups into BCOLS-col batches, per band
    batches = []
    colmap = np.full((NBAND, G), -1, np.int64)
    batch_of_group = np.full((NBAND, G), -1, np.int64)
    for b in range(NBAND):
        blist = []
        cur = []
        used = 0
        for g in range(G):
            r = int(R[b, g])
            if r == 0:
                break
            if used + r > BCOLS:
                blist.append(cur)
                cur = []
                used = 0
            if cur and cur[-1][3] == r and cur[-1][2] == g:
                lcol0, g0, g1, rr = cur[-1]
                cur[-1] = (lcol0, g0, g + 1, rr)
            else:
                cur.append((used, g, g + 1, r))
            colmap[b, g] = used
            batch_of_group[b, g] = len(blist)
            used += r
        if cur:
            blist.append(cur)
        batches.append(blist)

    nbt_band = [len(batches[b]) for b in range(NBAND)]
    NBT = sum(nbt_band)
    batch_band = []
    batch_levels = []
    for b in range(NBAND):
        for bb in batches[b]:
            batch_band.append(b)
            batch_levels.append(bb)
    batch_base = np.cumsum([0] + nbt_band)

    gidx = []
    for c in range(ncore):
        stream = np.full(NBT * BATCH, ZROW, np.int64)
        for b in range(NBAND):
            m = (core == c) & (band == b)
            r = rank[c, b][ldst[m]]
            v = row[m]
            o = np.argsort(r, kind="stable")
            r = r[o]
            v = v[o]
            starts = np.searchsorted(r, ar)
            k = np.arange(len(r)) - starts[r]
            g = r // P
            p = r % P
            gb = batch_base[b] + batch_of_group[b, g]
            col = colmap[b, g] + k
            pos = gb * BATCH + col * P + p
            stream[pos] = v
        gidx.append(_wrap_idx(stream))

    return {
        "ncore": ncore,
        "dpc": dpc,
        "G": G,
        "NBT": NBT,
        "batch_band": batch_band,
        "batch_levels": batch_levels,
        "batch_base": batch_base,
        "order": order,
        "gidx": gidx,
    }


def _build_nc(sched, reps=1):
    ncore = sched["ncore"]
    G = sched["G"]
    NBT = sched["NBT"]
    batch_band = sched["batch_band"]
    batch_levels = sched["batch_levels"]

    nc = bacc.Bacc("TRN2", target_bir_lowering=False, debug=False,
                   num_devices=ncore, num_swdge_queues=4,
                   dynamic_dma_scratch_size=DSS)

    tab = nc.dram_tensor("tab", [TROW, 128], mybir.dt.bfloat16,
                         kind="ExternalInput")
    gidx = nc.dram_tensor("gidx", [P, NBT * IDXC], mybir.dt.int16,
                          kind="ExternalInput")
    outp = nc.dram_tensor("outp", [NBAND, P, G * D], mybir.dt.bfloat16,
                          kind="ExternalOutput")

    gring = nc.alloc_sbuf_tensor("gring", [P, IDXSLOT * IDXC], mybir.dt.int16)
    stage = nc.alloc_sbuf_tensor("stage", [P, NSLOT * BCOLS * D],
                                 mybir.dt.bfloat16)
    acc = [nc.alloc_sbuf_tensor(f"acc{b}", [P, G * D], mybir.dt.bfloat16)
           for b in range(NBAND)]

    with (
        nc.Block() as block,
        nc.semaphore("ld") as ld,
        nc.semaphore("q0") as q0s,
        nc.semaphore("q1") as q1s,
        nc.semaphore("q2") as q2s,
        nc.semaphore("q3") as q3s,
        nc.semaphore("qp0") as qp0,
        nc.semaphore("qp1") as qp1,
        nc.semaphore("qp2") as qp2,
        nc.semaphore("qp3") as qp3,
        nc.semaphore("red") as red,
        nc.semaphore("wo") as wo,
    ):
        qdma = [q0s, q1s, q2s, q3s]
        qprep = [qp0, qp1, qp2, qp3]

        @block.sync
        def _(s: bass.BassEngine):
            for rep in range(reps):
                for k in range(NBT):
                    gk = rep * NBT + k
                    if gk >= IDXSLOT:
                        pk = gk - IDXSLOT
                        s.wait_ge(qprep[pk % 4], pk // 4 + 1)
                    s.dma_start(
                        out=gring[:, (gk % IDXSLOT) * IDXC:(gk % IDXSLOT + 1) * IDXC],
                        in_=gidx[:, k * IDXC:(k + 1) * IDXC],
                    ).then_inc(ld, 16)

        @block.gpsimd
        def _(g: bass.BassGpSimd):
            uses = [0] * 4
            pcnt = [0] * 4
            for rep in range(reps):
                for k in range(NBT):
                    gk = rep * NBT + k
                    q = gk % 4
                    b = batch_band[k]
                    g.wait_ge(ld, 16 * (gk + 1))
                    if gk >= NSLOT:
                        g.wait_ge(red, gk - NSLOT + 1)
                    if uses[q] >= 1:
                        g.wait_ge(qdma[q], 16 * uses[q])
                    _dma_gather_raw(
                        g,
                        out_ap=stage.ap().rearrange("p (c d) -> p c d", d=D)[
                            :, (gk % NSLOT) * BCOLS:(gk % NSLOT) * BCOLS + BCOLS, :],
                        in_ap=tab[0:TROW, 32 * b:32 * b + 32],
                        idxs_ap=gring[:, (gk % IDXSLOT) * IDXC:(gk % IDXSLOT + 1) * IDXC],
                        num_idxs=BATCH,
                        num_idxs_reg=BATCH,
                        elem_size=32,
                        elem_step=128,
                        sem=qdma[q],
                        queue_num=q,
                    ).then_inc(qprep[q], 1)
                    uses[q] += 1
                    pcnt[q] += 1
                    g.wait_ge(qprep[q], pcnt[q])
                    g.trigger_dma(count=1, queue_num=q)
            for q in range(4):
                if uses[q]:
                    g.wait_ge(qdma[q], 16 * uses[q])

        @block.vector
        def _(v: bass.BassEngine):
            with nc.allow_low_precision("bf16 partials; host sums in f32"):
                for b in range(NBAND):
                    v.memset(acc[b].ap(), 0.0)
                for rep in range(reps):
                    for k in range(NBT):
                        gk = rep * NBT + k
                        b = batch_band[k]
                        v.wait_ge(qdma[gk % 4], 16 * (gk // 4 + 1))
                        slot0 = (gk % NSLOT) * BCOLS
                        acc_ap = acc[b].ap().rearrange("p (g d) -> p g d", d=D)
                        last = None
                        for (lcol0, g0, g1, R) in batch_levels[k]:
                            src = stage.ap().rearrange("p (c d) -> p c d", d=D)[
                                :, slot0 + lcol0: slot0 + lcol0 + (g1 - g0) * R, :] \
                                .rearrange("p (g r) d -> p g d r", r=R)
                            last = v.tensor_reduce(
                                out=acc_ap[:, g0:g1, :],
                                in_=src,
                                axis=mybir.AxisListType.X,
                                op=mybir.AluOpType.add,
                            )
                        last.then_inc(red, 1)

        @block.scalar
        def _(s: bass.BassEngine):
            nwo = 0
            for rep in range(reps):
                for b in range(NBAND):
                    lk = sched["batch_base"][b + 1] - 1
                    s.wait_ge(red, rep * NBT + lk + 1)
                    s.dma_start(out=outp[b], in_=acc[b][:]).then_inc(wo, 16)
                    nwo += 1
            s.wait_ge(wo, 16 * nwo)

    nc.compile()
    return nc


def _prep_inputs(features, src, dst, ncore=NCORE):
    features = np.asarray(features, np.float32)
    src32 = np.asarray(src).astype(np.int32)
    dst32 = np.asarray(dst).astype(np.int32)
    sched = _build_schedule(src32, dst32, ncore)

    fpad = np.zeros((NPAD, D), np.float32)
    fpad[:N_NODES] = features
    tabf = np.zeros((TROW, 128), np.float32)
    tabf[:ZROW] = fpad.reshape(ZROW, 128)
    tab = tabf.astype(ml_dtypes.bfloat16)

    in_maps = [{"tab": tab, "gidx": sched["gidx"][c]}
               for c in range(sched["ncore"])]
    return sched, in_maps


def _run(nc, in_maps):
    try:
        return run_bass_kernel_spmd(nc, in_maps, list(range(len(in_maps))))
    except Exception:
        return run_bass_kernel_spmd(nc, in_maps, list(range(len(in_maps))))


def _postprocess(res, sched):
    ncore = sched["ncore"]
    dpc = sched["dpc"]
    G = sched["G"]
    order = sched["order"]
    out = np.zeros((NPAD, D), np.float32)
    for c in range(ncore):
        o = res.results[c]["outp"]  # [NBAND, P, G*D] bf16
        for b in range(NBAND):
            vals = np.asarray(o[b], dtype=np.float32).reshape(P, G, D)
            vals = vals.transpose(1, 0, 2).reshape(G * P, D)  # rank-major
            out[c * dpc + order[c, b]] += vals
    return np.ascontiguousarray(out[:N_NODES])


def kernel(features, src, dst):
    sched, in_maps = _prep_inputs(features, src, dst)
    nc = _build_nc(sched)
    res = _run(nc, in_maps)
    return _postprocess(res, sched)


if __name__ == "__main__":
    rng = np.random.default_rng(0)
    feats = rng.standard_normal((N_NODES, D)).astype(np.float32)
    src = rng.integers(0, N_NODES, N_EDGES).astype(np.int64)
    dst = rng.integers(0, N_NODES, N_EDGES).astype(np.int64)
    got = kernel(feats, src, dst)
    exp = np.zeros((N_NODES, D), np.float32)
    np.add.at(exp, dst, feats[src])
    err = np.linalg.norm(got - exp) / np.linalg.norm(exp)
    print("rel err:", err)


# revision 3
# speedup vs baseline: 25.0937x; 2.5241x over previous
"""GCN message-passing (gather + segment-sum) on TRN2.

out[v] = sum over edges (u -> v) of features[u]
(features [100000, 32] f32, 1.6M edges)

Empirical facts for this environment (axon-tunneled TRN2):
  - DMA throughput collapses when all 8 cores issue heavy descriptor
    traffic concurrently; 1-4 cores run ~2 orders of magnitude faster
    per descriptor.  So we use 4 cores.
  - dma_gather handles 64-byte elements fine (bass's 256B-multiple
    assert is a front-end restriction; a relaxed clone of the call is
    inlined below).  Row pitch must still be a 256B multiple.
  - Large DVE ops are nearly free; small instructions cost ~1-3us, so
    everything is batched (4096-idx gathers, merged reduce levels).

Design:
  - dst-sharded across 4 cores; core c owns local dst range of 25088.
  - Features bf16, packed 4 nodes per 256B table row: node u lives at
    row u//4, col 32*(u%4); one zero row for padding.  The table stays
    in DRAM; gathers move only 64B per edge.
  - 4 "band" streams per core (band = src%4 = the col offset of the
    gather's in_ap).  Per (core, band), dst slots are ranked by edge
    count (desc); groups of 128 ranks share a run length R
    (cross-core max -> one static NEFF serves all cores).  Whole groups
    are packed into 64-col gather batches.
  - SWDGE gathers (8192 idxs/batch, 4 queues, staging ring) feed DVE
    tensor_reduce (strided X) which writes bf16 partial sums
    acc_b[128, G*32] (slot rank r -> partition r%128, group r//128).
  - One plain DMA per band writes acc_b out; the host applies the
    rank->node permutations and sums the 4 band partials in f32.
"""

import numpy as np
import ml_dtypes

import concourse.bass as bass
import concourse.mybir as mybir
from concourse import ap_utils
from concourse import bacc
from concourse.bass import MemorySpace, exact_div
from concourse.bass_utils import run_bass_kernel_spmd

N_NODES = 100000
N_EDGES = 1600000
D = 32
P = 128
NPAD = 100352          # 784 * 128
NBAND = 4
TROW = 25089           # 25088 packed rows + 1 zero row
ZROW = 25088

NCORE = 4
BATCH = 8192
BCOLS = BATCH // P     # 64 gather cols per batch
IDXC = BATCH // 16     # 512 wrapped idx cols per batch
DSS = 98304
NSLOT = 6              # staging ring slots (each BCOLS cols)
IDXSLOT = 8            # gidx ring slots


def _dma_gather_raw(g, out_ap, in_ap, idxs_ap, num_idxs, num_idxs_reg,
                    elem_size, elem_step, sem, queue_num):
    """bass.BassGpSimd.dma_gather (non-transpose, HBM source,
    prepare_only) with the elem_size_bytes % 256 assert relaxed to 64."""
    self = g
    self._assert_queue_num(queue_num)
    assert idxs_ap.dtype == mybir.dt.int16
    assert in_ap.dtype == out_ap.dtype
    elem_size_bytes = elem_size * mybir.dt.size(in_ap.dtype)
    assert elem_size_bytes > 0 and elem_size_bytes % 64 == 0
    assert in_ap.space == MemorySpace.DRAM
    assert idxs_ap.space == MemorySpace.SBUF
    assert out_ap.space == MemorySpace.SBUF
    assert ap_utils.ap_is_contiguous(out_ap.ap[1:])
    assert ap_utils.ap_is_contiguous(idxs_ap.ap[1:])
    assert in_ap.ap[-1][1] == out_ap.ap[-1][1] == elem_size
    assert out_ap.ap[0][1] * out_ap.ap[1][1] == bass.round_up_to_multiple(
        num_idxs, 128)
    assert in_ap.ap[0][0] == elem_step
    stride_bytes = elem_step * mybir.dt.size(in_ap.dtype)
    stride_bytes_256 = exact_div(stride_bytes, 256)
    assert stride_bytes_256 < 256

    _in_ap = self.lower_ap_dma(in_ap, for_custom_bir_dma=True)
    inst = self.add_instruction(
        mybir.InstDMAGatherAnt(
            name=self.bass.get_next_instruction_name(),
            ins=[
                *_in_ap,
                self.lower_ap(idxs_ap),
                self.lower_val_access(self.to_reg(num_idxs_reg)),
            ],
            outs=[self.lower_ap(out_ap)],
            transpose=False,
            num_idxs=num_idxs,
            elem_size=elem_size,
            stride_bytes_256=stride_bytes_256,
            gen_mode=1,
            single_packet=False,
            queue_num=queue_num,
            sbuf_tokens_per_rank=0,
            sbuf_free_dim_per_rank=0,
            sbuf_free_dim_pad_per_rank=0,
            sbuf_byte_offset=0,
        )
    )
    inst.then_inc(sem, 16)
    return self._track_prepare_only(inst, queue_num)


def _wrap_idx(stream):
    n = len(stream)
    w = np.asarray(stream, np.int16).reshape(n // 16, 16).T
    return np.tile(w, (8, 1))


def _build_schedule(src32, dst32, ncore=NCORE):
    dpc = NPAD // ncore
    G = dpc // P
    core = dst32 // dpc
    ldst = dst32 - core * dpc
    band = src32 % NBAND
    row = src32 // NBAND

    flat = (core * NBAND + band) * dpc + ldst
    cnt = np.bincount(flat, minlength=ncore * NBAND * dpc)
    cnt = cnt.reshape(ncore, NBAND, dpc).astype(np.int32)

    order = np.argsort(-cnt, axis=2, kind="stable")     # rank -> ldst
    scnt = -np.sort(-cnt, axis=2)

    # shared group run lengths: cross-core max at group heads
    R = scnt[:, :, 0::P].max(axis=0)                    # [NBAND, G]
    assert R.max() <= BCOLS, f"R.max()={R.max()} exceeds batch cols {BCOLS}"

    rank = np.empty_like(order)
    ar = np.arange(dpc)
    for c in range(ncore):
        for b in range(NBAND):
            rank[c, b, order[c, b]] = ar

    # pack rank-groups into BCOLS-col batches, per band
    batches = []
    colmap = np.full((NBAND, G), -1, np.int64)
    batch_of_group = np.full((NBAND, G), -1, np.int64)
    for b in range(NBAND):
        blist = []
        cur = []
        used = 0
        for g in range(G):
            r = int(R[b, g])
            if r == 0:
                break
            if used + r > BCOLS:
                blist.append(cur)
                cur = []
                used = 0
            if cur and cur[-1][3] == r and cur[-1][2] == g:
                lcol0, g0, g1, rr = cur[-1]
                cur[-1] = (lcol0, g0, g + 1, rr)
            else:
                cur.append((used, g, g + 1, r))
            colmap[b, g] = used
            batch_of_group[b, g] = len(blist)
            used += r
        if cur:
            blist.append(cur)
        batches.append(blist)

    nbt_band = [len(batches[b]) for b in range(NBAND)]
    NBT = sum(nbt_band)
    batch_band = []
    batch_levels = []
    for b in range(NBAND):
        for bb in batches[b]:
            batch_band.append(b)
            batch_levels.append(bb)
    batch_base = np.cumsum([0] + nbt_band)

    gidx = []
    for c in range(ncore):
        stream = np.full(NBT * BATCH, ZROW, np.int64)
        for b in range(NBAND):
            m = (core == c) & (band == b)
            r = rank[c, b][ldst[m]]
            v = row[m]
            o = np.argsort(r, kind="stable")
            r = r[o]
            v = v[o]
            starts = np.searchsorted(r, ar)
            k = np.arange(len(r)) - starts[r]
            g = r // P
            p = r % P
            gb = batch_base[b] + batch_of_group[b, g]
            col = colmap[b, g] + k
            pos = gb * BATCH + col * P + p
            stream[pos] = v
        gidx.append(_wrap_idx(stream))

    return {
        "ncore": ncore,
        "dpc": dpc,
        "G": G,
        "NBT": NBT,
        "batch_band": batch_band,
        "batch_levels": batch_levels,
        "batch_base": batch_base,
        "order": order,
        "gidx": gidx,
    }


def _build_nc(sched, reps=1):
    ncore = sched["ncore"]
    G = sched["G"]
    NBT = sched["NBT"]
    batch_band = sched["batch_band"]
    batch_levels = sched["batch_levels"]

    nc = bacc.Bacc("TRN2", target_bir_lowering=False, debug=False,
                   num_devices=ncore, num_swdge_queues=4,
                   dynamic_dma_scratch_size=DSS)

    tab = nc.dram_tensor("tab", [TROW, 128], mybir.dt.bfloat16,
                         kind="ExternalInput")
    gidx = nc.dram_tensor("gidx", [P, NBT * IDXC], mybir.dt.int16,
                          kind="ExternalInput")
    outp = nc.dram_tensor("outp", [NBAND, P, G * D], mybir.dt.bfloat16,
                          kind="ExternalOutput")

    gring = nc.alloc_sbuf_tensor("gring", [P, IDXSLOT * IDXC], mybir.dt.int16)
    stage = nc.alloc_sbuf_tensor("stage", [P, NSLOT * BCOLS * D],
                                 mybir.dt.bfloat16)
    acc = [nc.alloc_sbuf_tensor(f"acc{b}", [P, G * D], mybir.dt.bfloat16)
           for b in range(NBAND)]

    with (
        nc.Block() as block,
        nc.semaphore("ld") as ld,
        nc.semaphore("q0") as q0s,
        nc.semaphore("q1") as q1s,
        nc.semaphore("q2") as q2s,
        nc.semaphore("q3") as q3s,
        nc.semaphore("qp0") as qp0,
        nc.semaphore("qp1") as qp1,
        nc.semaphore("qp2") as qp2,
        nc.semaphore("qp3") as qp3,
        nc.semaphore("red") as red,
        nc.semaphore("wo") as wo,
    ):
        qdma = [q0s, q1s, q2s, q3s]
        qprep = [qp0, qp1, qp2, qp3]

        @block.sync
        def _(s: bass.BassEngine):
            for rep in range(reps):
                for k in range(NBT):
                    gk = rep * NBT + k
                    if gk >= IDXSLOT:
                        pk = gk - IDXSLOT
                        s.wait_ge(qprep[pk % 4], pk // 4 + 1)
                    s.dma_start(
                        out=gring[:, (gk % IDXSLOT) * IDXC:(gk % IDXSLOT + 1) * IDXC],
                        in_=gidx[:, k * IDXC:(k + 1) * IDXC],
                    ).then_inc(ld, 16)

        @block.gpsimd
        def _(g: bass.BassGpSimd):
            uses = [0] * 4
            pcnt = [0] * 4
            for rep in range(reps):
                for k in range(NBT):
                    gk = rep * NBT + k
                    q = gk % 4
                    b = batch_band[k]
                    g.wait_ge(ld, 16 * (gk + 1))
                    if gk >= NSLOT:
                        g.wait_ge(red, gk - NSLOT + 1)
                    if uses[q] >= 1:
                        g.wait_ge(qdma[q], 16 * uses[q])
                    _dma_gather_raw(
                        g,
                        out_ap=stage.ap().rearrange("p (c d) -> p c d", d=D)[
                            :, (gk % NSLOT) * BCOLS:(gk % NSLOT) * BCOLS + BCOLS, :],
                        in_ap=tab[0:TROW, 32 * b:32 * b + 32],
                        idxs_ap=gring[:, (gk % IDXSLOT) * IDXC:(gk % IDXSLOT + 1) * IDXC],
                        num_idxs=BATCH,
                        num_idxs_reg=BATCH,
                        elem_size=32,
                        elem_step=128,
                        sem=qdma[q],
                        queue_num=q,
                    ).then_inc(qprep[q], 1)
                    uses[q] += 1
                    pcnt[q] += 1
                    g.wait_ge(qprep[q], pcnt[q])
                    g.trigger_dma(count=1, queue_num=q)
            for q in range(4):
                if uses[q]:
                    g.wait_ge(qdma[q], 16 * uses[q])

        @block.vector
        def _(v: bass.BassEngine):
            with nc.allow_low_precision("bf16 partials; host sums in f32"):
                for b in range(NBAND):
                    v.memset(acc[b].ap(), 0.0)
                for rep in range(reps):
                    for k in range(NBT):
                        gk = rep * NBT + k
                        b = batch_band[k]
                        v.wait_ge(qdma[gk % 4], 16 * (gk // 4 + 1))
                        slot0 = (gk % NSLOT) * BCOLS
                        acc_ap = acc[b].ap().rearrange("p (g d) -> p g d", d=D)
                        last = None
                        for (lcol0, g0, g1, R) in batch_levels[k]:
                            src = stage.ap().rearrange("p (c d) -> p c d", d=D)[
                                :, slot0 + lcol0: slot0 + lcol0 + (g1 - g0) * R, :] \
                                .rearrange("p (g r) d -> p g d r", r=R)
                            last = v.tensor_reduce(
                                out=acc_ap[:, g0:g1, :],
                                in_=src,
                                axis=mybir.AxisListType.X,
                                op=mybir.AluOpType.add,
                            )
                        last.then_inc(red, 1)

        @block.scalar
        def _(s: bass.BassEngine):
            nwo = 0
            for rep in range(reps):
                for b in range(NBAND):
                    lk = sched["batch_base"][b + 1] - 1
                    s.wait_ge(red, rep * NBT + lk + 1)
                    s.dma_start(out=outp[b], in_=acc[b][:]).then_inc(wo, 16)
                    nwo += 1
            s.wait_ge(wo, 16 * nwo)

    nc.compile()
    return nc


def _prep_inputs(features, src, dst, ncore=NCORE):
    features = np.asarray(features, np.float32)
    src32 = np.asarray(src).astype(np.int32)
    dst32 = np.asarray(dst).astype(np.int32)
    sched = _build_schedule(src32, dst32, ncore)

    fpad = np.zeros((NPAD, D), np.float32)
    fpad[:N_NODES] = features
    tabf = np.zeros((TROW, 128), np.float32)
    tabf[:ZROW] = fpad.reshape(ZROW, 128)
    tab = tabf.astype(ml_dtypes.bfloat16)

    in_maps = [{"tab": tab, "gidx": sched["gidx"][c]}
               for c in range(sched["ncore"])]
    return sched, in_maps


def _run(nc, in_maps):
    try:
        return run_bass_kernel_spmd(nc, in_maps, list(range(len(in_maps))))
    except Exception:
        return run_bass_kernel_spmd(nc, in_maps, list(range(len(in_maps))))


def _postprocess(res, sched):
    ncore = sched["ncore"]
    dpc = sched["dpc"]
    G = sched["G"]
    order = sched["order"]
    out = np.zeros((NPAD, D), np.float32)
    for c in range(ncore):
        o = res.results[c]["outp"]  # [NBAND, P, G*D] bf16
        for b in range(NBAND):
            vals = np.asarray(o[b], dtype=np.float32).reshape(P, G, D)
            vals = vals.transpose(1, 0, 2).reshape(G * P, D)  # rank-major
            out[c * dpc + order[c, b]] += vals
    return np.ascontiguousarray(out[:N_NODES])


def kernel(features, src, dst):
    sched, in_maps = _prep_inputs(features, src, dst)
    nc = _build_nc(sched)
    res = _run(nc, in_maps)
    return _postprocess(res, sched)


if __name__ == "__main__":
    rng = np.random.default_rng(0)
    feats = rng.standard_normal((N_NODES, D)).astype(np.float32)
    src = rng.integers(0, N_NODES, N_EDGES).astype(np.int64)
    dst = rng.integers(0, N_NODES, N_EDGES).astype(np.int64)
    got = kernel(feats, src, dst)
    exp = np.zeros((N_NODES, D), np.float32)
    np.add.at(exp, dst, feats[src])
    err = np.linalg.norm(got - exp) / np.linalg.norm(exp)
    print("rel err:", err)
